# Initial kernel scaffold
#
"""Multi-head attention (H=16, D=1024, B=2, L=2048) on 8 TRN2 NeuronCores.

Sharding: head-parallel tensor parallelism per the hint. Core m owns heads
{2m, 2m+1} for both batches: Wq/Wk/Wv column-sharded (128 cols each),
Wo row-sharded (128 rows each). Each core reads full x/y (transposed on
host to [D, T] so the contraction dim lands on SBUF partitions), computes
its 4 (head, batch) attention panes flash-style, and writes a full-size
partial output; partials are summed on the host (the Wo row-shard reduce).

On-device layout is the "transposed domain" throughout:
  qT/kT [dh*2, T], S^T [keys, q], P^T [keys, q], ctx^T [dh, q], out^T [D, T].
Softmax denominators come free from a ones-column appended to V; the
per-query reciprocal is broadcast across partitions with a K=1 matmul.
All matmuls run in bf16 (f32 PSUM accumulation); the mask is shipped as a
multiplicative bf16 {0,1} tensor so the DVE mask-multiply runs in 2x mode.
"""

import sys

if "/opt/trn_rl_repo" not in sys.path:
    sys.path.insert(0, "/opt/trn_rl_repo")

import contextlib

import ml_dtypes
import numpy as np

import concourse.bass as bass
import concourse.mybir as mybir
import concourse.tile as tile

BF16 = mybir.dt.bfloat16
F32 = mybir.dt.float32
AF = mybir.ActivationFunctionType

H, D, B, L = 16, 1024, 2, 2048
T = B * L            # 4096 tokens, batch-major
DH = 64              # head dim
NCORES = 8
HL = H // NCORES     # 2 heads per core
SCALE = 1.0 / np.sqrt(D / H)  # 0.125
P1 = 128
NT = T // 512        # 8 column tiles of 512 tokens
DT = D // P1         # 8 contraction tiles
QT = 4               # query tiles per batch (512 each)
KT = 16              # key tiles per batch (128 each)
VW = 2 * (DH + 1)    # 130 cols per keytile in v_sb: [v_h0 | 1 | v_h1 | 1]

_CACHE = {}


def _build():
    nc = bass.Bass()
    xT = nc.dram_tensor("xT", [D, T], BF16, kind="ExternalInput")
    yT = nc.dram_tensor("yT", [D, T], BF16, kind="ExternalInput")
    wq = nc.dram_tensor("wq", [D, P1], BF16, kind="ExternalInput")
    wk = nc.dram_tensor("wk", [D, P1], BF16, kind="ExternalInput")
    wv = nc.dram_tensor("wv", [D, P1], BF16, kind="ExternalInput")
    wo = nc.dram_tensor("wo", [P1, D], BF16, kind="ExternalInput")
    maskT = nc.dram_tensor("maskT", [2 * HL, L, L], BF16, kind="ExternalInput")
    outT = nc.dram_tensor("outT", [D, T], F32, kind="ExternalOutput")

    with tile.TileContext(nc) as tc, contextlib.ExitStack() as ctx:
        singles = ctx.enter_context(tc.tile_pool(name="singles", bufs=1))
        wq_sb = singles.tile([P1, DT, P1], BF16)
        wk_sb = singles.tile([P1, DT, P1], BF16)
        wv_sb = singles.tile([P1, DT, P1], BF16)
        wo_sb = singles.tile([P1, D], BF16)
        qT_sb = singles.tile([P1, T], BF16)
        kT_sb = singles.tile([P1, T], BF16)
        v_sb = singles.tile([P1, 2 * KT * VW], BF16)
        ones_sb = singles.tile([P1, DH], BF16)

        nc.sync.dma_start(out=wq_sb, in_=wq[:].rearrange("(t p) c -> p t c", p=P1))
        nc.sync.dma_start(out=wk_sb, in_=wk[:].rearrange("(t p) c -> p t c", p=P1))
        nc.sync.dma_start(out=wv_sb, in_=wv[:].rearrange("(t p) c -> p t c", p=P1))
        nc.sync.dma_start(out=wo_sb, in_=wo[:])
        nc.vector.memset(v_sb, 1.0)   # ones columns survive; v data overwritten
        nc.vector.memset(ones_sb, 1.0)

        # ---- Phase P: projections  qT/kT [128, T], v [T, 128] -------------
        with tc.tile_pool(name="pp", bufs=2, space="PSUM") as pp, \
             tc.tile_pool(name="xy", bufs=9) as xy:
            for nt in range(NT):
                c0 = nt * 512
                qp = pp.tile([P1, 512], F32, tag="qp", bufs=2)
                kp = pp.tile([P1, 512], F32, tag="kp", bufs=2)
                yts = []
                for dt_i in range(DT):
                    xt = xy.tile([P1, 512], BF16, tag="xt", bufs=3)
                    nc.sync.dma_start(out=xt, in_=xT[dt_i * P1:(dt_i + 1) * P1, c0:c0 + 512])
                    yt = xy.tile([P1, 512], BF16, tag="yt", bufs=9)
                    nc.sync.dma_start(out=yt, in_=yT[dt_i * P1:(dt_i + 1) * P1, c0:c0 + 512])
                    yts.append(yt)
                    nc.tensor.matmul(qp, lhsT=wq_sb[:, dt_i, :], rhs=xt,
                                     start=(dt_i == 0), stop=(dt_i == DT - 1))
                    nc.tensor.matmul(kp, lhsT=wk_sb[:, dt_i, :], rhs=yt,
                                     start=(dt_i == 0), stop=(dt_i == DT - 1))
                nc.vector.tensor_copy(out=qT_sb[:, c0:c0 + 512], in_=qp)
                nc.vector.tensor_copy(out=kT_sb[:, c0:c0 + 512], in_=kp)
                for tt in range(4):
                    vp = pp.tile([P1, P1], F32, tag="vp", bufs=4)
                    for dt_i in range(DT):
                        nc.tensor.matmul(vp, lhsT=yts[dt_i][:, tt * P1:(tt + 1) * P1],
                                         rhs=wv_sb[:, dt_i, :],
                                         start=(dt_i == 0), stop=(dt_i == DT - 1))
                    kv = (nt * 4 + tt) * VW
                    nc.vector.tensor_copy(out=v_sb[:, kv:kv + DH], in_=vp[:, 0:DH])
                    nc.vector.tensor_copy(out=v_sb[:, kv + DH + 1:kv + 2 * DH + 1],
                                          in_=vp[:, DH:2 * DH])

        # ---- Phase A/O: attention + output projection ----------------------
        with tc.tile_pool(name="pa", bufs=2, space="PSUM") as pa, \
             tc.tile_pool(name="mp", bufs=3) as mp, \
             tc.tile_pool(name="ptp", bufs=3) as ptp, \
             tc.tile_pool(name="aux", bufs=2) as aux, \
             tc.tile_pool(name="cxp", bufs=10) as cxp, \
             tc.tile_pool(name="obp", bufs=4) as obp:
            for b in range(2):
                t0 = b * L
                ctxn = {}
                for h in range(HL):
                    hc = h * DH
                    ctxs = [pa.tile([DH + 1, 512], F32, tag="ctx", bufs=4)
                            for _ in range(QT)]
                    for kt2 in range(KT):
                        j = 2 * h + b
                        kv = (b * KT + kt2) * VW + h * (DH + 1)
                        mt = mp.tile([P1, L], BF16, tag="mt", bufs=3)
                        nc.sync.dma_start(out=mt, in_=maskT[j, kt2 * P1:(kt2 + 1) * P1, :])
                        pt = ptp.tile([P1, L], BF16, tag="pt", bufs=3)
                        for qt in range(QT):
                            sp = pa.tile([P1, 512], F32, tag="sp", bufs=2)
                            nc.tensor.matmul(
                                sp,
                                lhsT=kT_sb[hc:hc + DH, t0 + kt2 * P1:t0 + (kt2 + 1) * P1],
                                rhs=qT_sb[hc:hc + DH, t0 + qt * 512:t0 + (qt + 1) * 512],
                                start=True, stop=True)
                            nc.scalar.activation(out=pt[:, qt * 512:(qt + 1) * 512],
                                                 in_=sp, func=AF.Exp, scale=float(SCALE))
                        nc.vector.tensor_mul(pt, pt, mt)
                        for qt in range(QT):
                            nc.tensor.matmul(ctxs[qt], lhsT=v_sb[:, kv:kv + DH + 1],
                                             rhs=pt[:, qt * 512:(qt + 1) * 512],
                                             start=(kt2 == 0), stop=(kt2 == KT - 1))
                    for qt in range(QT):
                        rc = aux.tile([DH + 1, 512], BF16, tag="rc", bufs=2)
                        nc.vector.reciprocal(out=rc[DH:DH + 1, :], in_=ctxs[qt][DH:DH + 1, :])
                        bc = pa.tile([DH, 512], F32, tag="op", bufs=2)
                        nc.tensor.matmul(bc, lhsT=ones_sb[DH:DH + 1, :], rhs=rc[DH:DH + 1, :],
                                         start=True, stop=True)
                        bcs = aux.tile([DH, 512], BF16, tag="bcs", bufs=2)
                        nc.scalar.copy(out=bcs, in_=bc)
                        cx = cxp.tile([DH, 512], BF16, tag="cx", bufs=10)
                        nc.vector.tensor_mul(cx, ctxs[qt][0:DH, :], bcs)
                        ctxn[(h, qt)] = cx
                for qt in range(QT):
                    for ot in range(8):
                        op_ = pa.tile([P1, 512], F32, tag="op", bufs=2)
                        nc.tensor.matmul(op_, lhsT=wo_sb[0:DH, ot * P1:(ot + 1) * P1],
                                         rhs=ctxn[(0, qt)], start=True, stop=False,
                                         tile_position=(0, 0))
                        nc.tensor.matmul(op_, lhsT=wo_sb[DH:P1, ot * P1:(ot + 1) * P1],
                                         rhs=ctxn[(1, qt)], start=False, stop=True,
                                         tile_position=(0, 0))
                        ob = obp.tile([P1, 512], F32, tag="ob", bufs=4)
                        nc.vector.tensor_copy(out=ob, in_=op_)
                        nc.sync.dma_start(
                            out=outT[ot * P1:(ot + 1) * P1, t0 + qt * 512:t0 + (qt + 1) * 512],
                            in_=ob)
    return nc


def make_in_maps(x, y, mask, Wq, Wk, Wv, Wo):
    bf = ml_dtypes.bfloat16
    xT = np.ascontiguousarray(x.reshape(T, D).T).astype(bf)
    yT = np.ascontiguousarray(y.reshape(T, D).T).astype(bf)
    in_maps = []
    for m in range(NCORES):
        c = slice(P1 * m, P1 * (m + 1))
        in_maps.append({
            "xT": xT,
            "yT": yT,
            "wq": np.ascontiguousarray(Wq[:, c]).astype(bf),
            "wk": np.ascontiguousarray(Wk[:, c]).astype(bf),
            "wv": np.ascontiguousarray(Wv[:, c]).astype(bf),
            "wo": np.ascontiguousarray(Wo[c, :]).astype(bf),
            # rows 4m..4m+3 are (h0,b0),(h0,b1),(h1,b0),(h1,b1) = j=2h+b order
            "maskT": np.ascontiguousarray(
                mask[4 * m:4 * m + 4].transpose(0, 2, 1)).astype(bf),
        })
    return in_maps


def assemble_output(partials):
    acc = np.zeros((D, T), np.float32)
    for p in partials:
        acc += p
    return np.ascontiguousarray(acc.T).reshape(B, L, D).astype(np.float32)


def kernel(x, y, mask, Wq, Wk, Wv, Wo):
    from concourse.bass_utils import run_bass_kernel_spmd

    if "nc" not in _CACHE:
        _CACHE["nc"] = _build()
    nc = _CACHE["nc"]
    in_maps = make_in_maps(np.asarray(x), np.asarray(y), np.asarray(mask),
                           np.asarray(Wq), np.asarray(Wk), np.asarray(Wv),
                           np.asarray(Wo))
    res = run_bass_kernel_spmd(nc, in_maps, core_ids=list(range(NCORES)))
    return assemble_output([r["outT"] for r in res.results])


# revision 13
# speedup vs baseline: 2.0255x; 2.0255x over previous
"""Multi-head attention (H=16, D=1024, B=2, L=2048) on 8 TRN2 NeuronCores.

Sharding: head-parallel tensor parallelism per the hint. Core m owns heads
{2m, 2m+1} for both batches: Wq/Wk/Wv column-sharded (128 cols each),
Wo row-sharded (128 rows each). Each core reads full x/y (transposed on
host to [D, T] so the contraction dim lands on SBUF partitions), computes
its 4 (head, batch) attention panes flash-style, and writes a full-size
partial output; partials are summed on the host (the Wo row-shard reduce).

On-device layout is the "transposed domain" throughout:
  qT/kT [dh*2, T], S^T [keys, q], P^T [keys, q], ctx^T [dh, q], out^T [D, T].
Softmax denominators come free from a ones-column appended to V; the
per-query reciprocal is broadcast across partitions with a K=1 matmul.
All matmuls run in bf16 (f32 PSUM accumulation); the mask is shipped as a
multiplicative bf16 {0,1} tensor so the DVE mask-multiply runs in 2x mode.
"""

import sys

if "/opt/trn_rl_repo" not in sys.path:
    sys.path.insert(0, "/opt/trn_rl_repo")

import contextlib

import ml_dtypes
import numpy as np

import concourse.bass as bass
import concourse.mybir as mybir
import concourse.tile as tile
from concourse import bacc

BF16 = mybir.dt.bfloat16
F32 = mybir.dt.float32
AF = mybir.ActivationFunctionType

H, D, B, L = 16, 1024, 2, 2048
T = B * L            # 4096 tokens, batch-major
DH = 64              # head dim
NCORES = 8
HL = H // NCORES     # 2 heads per core
SCALE = 1.0 / np.sqrt(D / H)  # 0.125
P1 = 128
NT = T // 512        # 8 column tiles of 512 tokens
DT = D // P1         # 8 contraction tiles
QT = 4               # query tiles per batch (512 each)
KT = 16              # key tiles per batch (128 each)
VW = 2 * (DH + 1)    # 130 cols per keytile in v_sb: [v_h0 | 1 | v_h1 | 1]

_CACHE = {}


def _build(chain=False, reps=1):
    nc = bacc.Bacc("TRN2", target_bir_lowering=False, debug=False)
    if chain:
        tick = nc.dram_tensor("tick", [1, 1], F32, kind="ExternalInput")
        tock = nc.dram_tensor("tock", [1, 1], F32, kind="ExternalOutput")
    xT = nc.dram_tensor("xT", [D, T], BF16, kind="ExternalInput")
    yT = nc.dram_tensor("yT", [D, T], BF16, kind="ExternalInput")
    wq = nc.dram_tensor("wq", [D, P1], BF16, kind="ExternalInput")
    wk = nc.dram_tensor("wk", [D, P1], BF16, kind="ExternalInput")
    wv = nc.dram_tensor("wv", [D, P1], BF16, kind="ExternalInput")
    wo = nc.dram_tensor("wo", [P1, D], BF16, kind="ExternalInput")
    maskT = nc.dram_tensor("maskT", [2 * HL, L, L], BF16, kind="ExternalInput")
    outT = nc.dram_tensor("outT", [D, T], F32, kind="ExternalOutput")

    with tile.TileContext(nc) as tc, contextlib.ExitStack() as ctx:
        singles = ctx.enter_context(tc.tile_pool(name="singles", bufs=1))
        if chain:
            tick_sb = singles.tile([1, 1], F32)
            nc.sync.dma_start(out=tick_sb, in_=tick[:])
            nc.vector.tensor_scalar_add(tick_sb, tick_sb, 1.0)
            nc.sync.dma_start(out=tock[:], in_=tick_sb)
        wq_sb = singles.tile([P1, DT, P1], BF16)
        wk_sb = singles.tile([P1, DT, P1], BF16)
        wv_sb = singles.tile([P1, DT, P1], BF16)
        wo_lo = singles.tile([DH, D], BF16)
        wo_hi = singles.tile([DH, D], BF16)
        qT_sb = singles.tile([P1, T], BF16)
        kT_sb = singles.tile([P1, T], BF16)
        v_sb = singles.tile([P1, 2 * KT * VW], BF16)
        ones_sb = singles.tile([P1, DH], BF16)

        nc.sync.dma_start(out=wq_sb, in_=wq[:].rearrange("(t p) c -> p t c", p=P1))
        nc.sync.dma_start(out=wk_sb, in_=wk[:].rearrange("(t p) c -> p t c", p=P1))
        nc.sync.dma_start(out=wv_sb, in_=wv[:].rearrange("(t p) c -> p t c", p=P1))
        nc.sync.dma_start(out=wo_lo, in_=wo[0:DH, :])
        nc.sync.dma_start(out=wo_hi, in_=wo[DH:P1, :])
        nc.vector.memset(v_sb, 1.0)   # ones columns survive; v data overwritten
        nc.vector.memset(ones_sb, 1.0)

        for rep in range(reps):
            _emit_body(nc, tc, rep, xT, yT, maskT, outT, wq_sb, wk_sb, wv_sb,
                       wo_lo, wo_hi, qT_sb, kT_sb, v_sb, ones_sb)
    nc.compile()
    return nc


def _emit_body(nc, tc, rep, xT, yT, maskT, outT, wq_sb, wk_sb, wv_sb,
               wo_lo, wo_hi, qT_sb, kT_sb, v_sb, ones_sb):
    # ---- Phase P: projections  qT/kT [128, T], v [T, 128] -------------
    with tc.tile_pool(name=f"pp{rep}", bufs=2, space="PSUM") as pp, \
         tc.tile_pool(name=f"xy{rep}", bufs=9) as xy:
        for nt in range(NT):
            c0 = nt * 512
            qp = pp.tile([P1, 512], F32, tag="qp", bufs=2)
            kp = pp.tile([P1, 512], F32, tag="kp", bufs=2)
            yts = []
            for dt_i in range(DT):
                xt = xy.tile([P1, 512], BF16, tag="xt", bufs=8)
                nc.sync.dma_start(out=xt, in_=xT[dt_i * P1:(dt_i + 1) * P1, c0:c0 + 512])
                yt = xy.tile([P1, 512], BF16, tag="yt", bufs=9)
                nc.sync.dma_start(out=yt, in_=yT[dt_i * P1:(dt_i + 1) * P1, c0:c0 + 512])
                yts.append(yt)
                nc.tensor.matmul(qp, lhsT=wq_sb[:, dt_i, :], rhs=xt,
                                 start=(dt_i == 0), stop=(dt_i == DT - 1))
                nc.tensor.matmul(kp, lhsT=wk_sb[:, dt_i, :], rhs=yt,
                                 start=(dt_i == 0), stop=(dt_i == DT - 1))
            nc.vector.tensor_copy(out=qT_sb[:, c0:c0 + 512], in_=qp)
            nc.vector.tensor_copy(out=kT_sb[:, c0:c0 + 512], in_=kp)
            for tt in range(4):
                vp = pp.tile([P1, P1], F32, tag="vp", bufs=4)
                for dt_i in range(DT):
                    nc.tensor.matmul(vp, lhsT=yts[dt_i][:, tt * P1:(tt + 1) * P1],
                                     rhs=wv_sb[:, dt_i, :],
                                     start=(dt_i == 0), stop=(dt_i == DT - 1))
                kv = (nt * 4 + tt) * VW
                nc.vector.tensor_copy(out=v_sb[:, kv:kv + DH], in_=vp[:, 0:DH])
                nc.vector.tensor_copy(out=v_sb[:, kv + DH + 1:kv + 2 * DH + 1],
                                      in_=vp[:, DH:2 * DH])

    # ---- Phase A/O: attention + output projection ----------------------
    with tc.tile_pool(name=f"pa{rep}", bufs=2, space="PSUM") as pa, \
         tc.tile_pool(name=f"mp{rep}", bufs=3) as mp, \
         tc.tile_pool(name=f"ptp{rep}", bufs=3) as ptp, \
         tc.tile_pool(name=f"aux{rep}", bufs=2) as aux, \
         tc.tile_pool(name=f"cxp{rep}", bufs=10) as cxp, \
         tc.tile_pool(name=f"obp{rep}", bufs=4) as obp:
        for b in range(2):
            t0 = b * L
            ctxn = {}
            for h in range(HL):
                hc = h * DH
                ctxs = [pa.tile([DH + 1, 512], F32, tag="ctx", bufs=4,
                                name=f"ctx_{rep}_{b}_{h}_{i}")
                        for i in range(QT)]
                for kt2 in range(KT):
                    j = 2 * h + b
                    kv = (b * KT + kt2) * VW + h * (DH + 1)
                    mt = mp.tile([P1, L], BF16, tag="mt", bufs=3)
                    nc.sync.dma_start(out=mt, in_=maskT[j, kt2 * P1:(kt2 + 1) * P1, :])
                    pt = ptp.tile([P1, L], BF16, tag="pt", bufs=3)
                    for qt in range(QT):
                        sp = pa.tile([P1, 512], F32, tag="sp", bufs=2)
                        nc.tensor.matmul(
                            sp,
                            lhsT=kT_sb[hc:hc + DH, t0 + kt2 * P1:t0 + (kt2 + 1) * P1],
                            rhs=qT_sb[hc:hc + DH, t0 + qt * 512:t0 + (qt + 1) * 512],
                            start=True, stop=True)
                        nc.scalar.activation(out=pt[:, qt * 512:(qt + 1) * 512],
                                             in_=sp, func=AF.Exp, scale=float(SCALE))
                    nc.vector.tensor_mul(pt, pt, mt)
                    for qt in range(QT):
                        nc.tensor.matmul(ctxs[qt], lhsT=v_sb[:, kv:kv + DH + 1],
                                         rhs=pt[:, qt * 512:(qt + 1) * 512],
                                         start=(kt2 == 0), stop=(kt2 == KT - 1))
                for qt in range(QT):
                    rc = aux.tile([DH + 1, 512], BF16, tag="rc", bufs=2)
                    with nc.allow_low_precision(reason="softmax recip in bf16 is within tolerance"):
                        nc.vector.reciprocal(out=rc[DH:DH + 1, :], in_=ctxs[qt][DH:DH + 1, :])
                    bc = pa.tile([DH, 512], F32, tag="op", bufs=2)
                    nc.tensor.matmul(bc, lhsT=ones_sb[DH:DH + 1, :], rhs=rc[DH:DH + 1, :],
                                     start=True, stop=True)
                    bcs = aux.tile([DH, 512], BF16, tag="bcs", bufs=2)
                    nc.scalar.copy(out=bcs, in_=bc)
                    cx = cxp.tile([DH, 512], BF16, tag="cx", bufs=10)
                    nc.vector.tensor_mul(cx, ctxs[qt][0:DH, :], bcs)
                    ctxn[(h, qt)] = cx
            for qt in range(QT):
                for ot in range(8):
                    op_ = pa.tile([P1, 512], F32, tag="op", bufs=2)
                    nc.tensor.matmul(op_, lhsT=wo_lo[:, ot * P1:(ot + 1) * P1],
                                     rhs=ctxn[(0, qt)], start=True, stop=False)
                    nc.tensor.matmul(op_, lhsT=wo_hi[:, ot * P1:(ot + 1) * P1],
                                     rhs=ctxn[(1, qt)], start=False, stop=True)
                    ob = obp.tile([P1, 512], F32, tag="ob", bufs=4)
                    nc.vector.tensor_copy(out=ob, in_=op_)
                    nc.sync.dma_start(
                        out=outT[ot * P1:(ot + 1) * P1, t0 + qt * 512:t0 + (qt + 1) * 512],
                        in_=ob)


def make_in_maps(x, y, mask, Wq, Wk, Wv, Wo):
    bf = ml_dtypes.bfloat16
    xT = np.ascontiguousarray(x.reshape(T, D).T).astype(bf)
    yT = np.ascontiguousarray(y.reshape(T, D).T).astype(bf)
    in_maps = []
    for m in range(NCORES):
        c = slice(P1 * m, P1 * (m + 1))
        in_maps.append({
            "xT": xT,
            "yT": yT,
            "wq": np.ascontiguousarray(Wq[:, c]).astype(bf),
            "wk": np.ascontiguousarray(Wk[:, c]).astype(bf),
            "wv": np.ascontiguousarray(Wv[:, c]).astype(bf),
            "wo": np.ascontiguousarray(Wo[c, :]).astype(bf),
            # rows 4m..4m+3 are (h0,b0),(h0,b1),(h1,b0),(h1,b1) = j=2h+b order
            "maskT": np.ascontiguousarray(
                mask[4 * m:4 * m + 4].transpose(0, 2, 1)).astype(bf),
        })
    return in_maps


def assemble_output(partials):
    acc = np.zeros((D, T), np.float32)
    for p in partials:
        acc += p
    return np.ascontiguousarray(acc.T).reshape(B, L, D).astype(np.float32)


def kernel(x, y, mask, Wq, Wk, Wv, Wo):
    from concourse.bass_utils import run_bass_kernel_spmd

    if "nc" not in _CACHE:
        _CACHE["nc"] = _build()
    nc = _CACHE["nc"]
    in_maps = make_in_maps(np.asarray(x), np.asarray(y), np.asarray(mask),
                           np.asarray(Wq), np.asarray(Wk), np.asarray(Wv),
                           np.asarray(Wo))
    res = run_bass_kernel_spmd(nc, in_maps, core_ids=list(range(NCORES)))
    return assemble_output([r["outT"] for r in res.results])
